# revision 2
# baseline (speedup 1.0000x reference)
"""Trainium2 Bass kernel for nn_AMM_76647986364863 (retrieval_knn).

Strategy: data-parallel over the batch dim of x across 8 NeuronCores
(64 rows/core); all tables/encoders replicated. Zero collectives.

Per-core compute keeps every activation transposed (features on the
partition dim, batch=64 on the free dim) so the whole chain is
weight-stationary matmuls with no on-chip transposes:
    qT      = key_enc^T x^T          (lhsT = key_enc)
    alphasT = keys_t0 qT             (lhsT = keys_t0^T, host-transposed)
    ybT     = vals_t0^T betasT       (lhsT = vals_t0)
    z0T     = val_enc ybT            (lhsT = val_enc^T, host-transposed)
    ISTA:  uT = val_enc^T zT (lhsT = val_enc);  gT = val_enc rT (lhsT = val_enc^T)
    t1:    alphas2T = keys_t1^T xT (lhsT = keys_t1); y1T = vals_t1 b2T (lhsT = vals_t1^T)

Since val_enc has exactly orthonormal columns (val_enc^T val_enc = I),
ISTA iteration 1 reduces to z1 = soft(z0): the first pair of big
matmuls is folded away (exact-math equivalent to the reference).

Matmuls run in bf16 (fp32 PSUM accumulate); weights are cast to bf16
on the host, halving HBM traffic. soft(x) = x - clamp(x, -1, 1) via a
fused tensor_scalar(max,min) + tensor_tensor(sub).
"""

import numpy as np

N = 2048      # x_dim
M = 2048      # y_dim
R0 = 1024
R1 = 1024
DK = 1024
DV = 1024
BATCH = 512
NCORES = 8
B = BATCH // NCORES            # 64 batch rows per core
ISTA_FULL_ITERS = 4            # reference does 5; iter 1 folds into soft(z0)

_CACHE = {}


def _build():
    from contextlib import ExitStack
    import concourse.tile as tile
    from concourse import bacc, mybir

    BF = mybir.dt.bfloat16
    F32 = mybir.dt.float32
    ALU = mybir.AluOpType

    nc = bacc.Bacc("TRN2", target_bir_lowering=False, debug=False,
                   num_devices=NCORES)

    def dp(name, shape, dt):
        return nc.dram_tensor(name, shape, dt, kind="ExternalInput").ap()

    xT_d = dp("xT", [N, B], BF)
    key_enc_d = dp("key_enc", [N, DK], BF)
    kt0T_d = dp("kt0T", [DK, R0], BF)
    vals_t0_d = dp("vals_t0", [R0, DV], BF)
    val_encT_d = dp("val_encT", [DV, M], BF)
    val_enc_d = dp("val_enc", [M, DV], BF)
    keys_t1_d = dp("keys_t1", [N, R1], BF)
    vt1T_d = dp("vt1T", [R1, M], BF)
    s0_d = dp("s0", [128, R0 // 128], F32)
    s1_d = dp("s1", [128, R1 // 128], F32)
    out_d = nc.dram_tensor("out", [M, B], F32, kind="ExternalOutput").ap()

    with tile.TileContext(nc) as tc, ExitStack() as ctx:
        wres = ctx.enter_context(tc.tile_pool(name="wres", bufs=1))
        wstream = ctx.enter_context(tc.tile_pool(name="wstream", bufs=2))
        acts = ctx.enter_context(tc.tile_pool(name="acts", bufs=1))
        psum = ctx.enter_context(tc.tile_pool(name="psum", bufs=8, space="PSUM"))

        def load_w(pool, ap, tag):
            K, F = ap.shape
            t = K // 128
            tl = pool.tile([128, t * F], BF, tag=tag, name=tag + "_w")
            nc.sync.dma_start(
                tl[:].rearrange("p (t f) -> p t f", t=t),
                ap.rearrange("(t p) f -> p t f", p=128),
            )
            return tl, t, F

        def act_tile(tag, nfree, dt=BF):
            return acts.tile([128, nfree], dt, tag=tag, name=tag)

        # ---- input / scale loads ----
        xT_sb = act_tile("xT", (N // 128) * B)
        nc.sync.dma_start(
            xT_sb[:].rearrange("p (t b) -> p t b", t=N // 128),
            xT_d.rearrange("(t p) b -> p t b", p=128),
        )
        s0_sb = act_tile("s0", R0 // 128, F32)
        nc.sync.dma_start(s0_sb[:], s0_d[:])
        s1_sb = act_tile("s1", R1 // 128, F32)
        nc.sync.dma_start(s1_sb[:], s1_d[:])

        def mm_layer(w_tl, t, F, rhs_tl, rhs_t, consumer):
            """out[m-tile] = sum_k lhsT[k,m-tile]^T @ rhs[k] for each of
            F//128 output tiles; consumer(m, psum_tile) evicts."""
            wv = w_tl[:].rearrange("p (t f) -> p t f", t=t)
            rv = rhs_tl[:].rearrange("p (t b) -> p t b", t=rhs_t)
            for m in range(F // 128):
                ps = psum.tile([128, B], F32, tag="ps", name="ps")
                for k in range(t):
                    nc.tensor.matmul(
                        ps[:],
                        wv[:, k, m * 128:(m + 1) * 128],
                        rv[:, k, :],
                        start=(k == 0),
                        stop=(k == t - 1),
                    )
                consumer(m, ps)

        # ---- table 0: q = x @ key_enc ----
        w_key, t_key, f_key = load_w(wstream, key_enc_d, "w")
        qT_sb = act_tile("qT", (DK // 128) * B)

        def ev_q(m, ps):
            nc.vector.tensor_copy(qT_sb[:, m * B:(m + 1) * B], ps[:])

        mm_layer(w_key, t_key, f_key, xT_sb, N // 128, ev_q)

        # ---- alphas/betas: betasT = (keys_t0 @ qT) * s0 ----
        w_kt0, t_kt0, f_kt0 = load_w(wstream, kt0T_d, "w")
        betasT_sb = act_tile("betasT", (R0 // 128) * B)

        def ev_beta(m, ps):
            nc.vector.tensor_scalar_mul(
                betasT_sb[:, m * B:(m + 1) * B], ps[:], s0_sb[:, m:m + 1]
            )

        mm_layer(w_kt0, t_kt0, f_kt0, qT_sb, DK // 128, ev_beta)

        # ---- ybT = vals_t0^T @ betasT ----
        w_v0, t_v0, f_v0 = load_w(wstream, vals_t0_d, "w")
        ybT_sb = act_tile("ybT", (DV // 128) * B)

        def ev_yb(m, ps):
            nc.vector.tensor_copy(ybT_sb[:, m * B:(m + 1) * B], ps[:])

        mm_layer(w_v0, t_v0, f_v0, betasT_sb, R0 // 128, ev_yb)

        # ---- z0 = yb @ val_enc^T ; z1 = soft(z0) ----
        w_veT, t_veT, f_veT = load_w(wres, val_encT_d, "val_encT")
        w_ve, t_ve, f_ve = load_w(wres, val_enc_d, "val_enc")
        zT_sb = act_tile("zT", (M // 128) * B)
        t_sb = act_tile("t", (M // 128) * B, F32)
        c_sb = act_tile("c", (M // 128) * B, F32)
        r_sb = act_tile("r", (DV // 128) * B)
        out_sb = act_tile("out", (M // 128) * B, F32)

        def ev_z0(m, ps):
            sl = slice(m * B, (m + 1) * B)
            nc.vector.tensor_scalar(
                c_sb[:, sl], ps[:], -1.0, 1.0, ALU.max, ALU.min
            )
            nc.vector.tensor_sub(zT_sb[:, sl], ps[:], c_sb[:, sl])

        mm_layer(w_veT, t_veT, f_veT, ybT_sb, DV // 128, ev_z0)

        # ---- ISTA full iterations ----
        for it in range(ISTA_FULL_ITERS):
            last = it == ISTA_FULL_ITERS - 1

            def ev_r(m, ps):
                sl = slice(m * B, (m + 1) * B)
                nc.vector.tensor_sub(r_sb[:, sl], ybT_sb[:, sl], ps[:])

            mm_layer(w_ve, t_ve, f_ve, zT_sb, M // 128, ev_r)

            def ev_soft(m, ps, last=last):
                sl = slice(m * B, (m + 1) * B)
                nc.vector.tensor_add(t_sb[:, sl], zT_sb[:, sl], ps[:])
                nc.vector.tensor_scalar(
                    c_sb[:, sl], t_sb[:, sl], -1.0, 1.0, ALU.max, ALU.min
                )
                dst = out_sb if last else zT_sb
                nc.vector.tensor_sub(dst[:, sl], t_sb[:, sl], c_sb[:, sl])

            mm_layer(w_veT, t_veT, f_veT, r_sb, DV // 128, ev_soft)

        # ---- table 1: y += (x @ keys_t1 * s1) @ vals_t1^T ----
        w_k1, t_k1, f_k1 = load_w(wstream, keys_t1_d, "w")
        b2_sb = act_tile("b2", (R1 // 128) * B)

        def ev_b2(m, ps):
            nc.vector.tensor_scalar_mul(
                b2_sb[:, m * B:(m + 1) * B], ps[:], s1_sb[:, m:m + 1]
            )

        mm_layer(w_k1, t_k1, f_k1, xT_sb, N // 128, ev_b2)

        w_v1T, t_v1T, f_v1T = load_w(wstream, vt1T_d, "w")

        def ev_y(m, ps):
            sl = slice(m * B, (m + 1) * B)
            nc.vector.tensor_add(out_sb[:, sl], out_sb[:, sl], ps[:])

        mm_layer(w_v1T, t_v1T, f_v1T, b2_sb, R1 // 128, ev_y)

        # ---- store ----
        nc.sync.dma_start(
            out_d.rearrange("(t p) b -> p t b", p=128),
            out_sb[:].rearrange("p (t b) -> p t b", t=M // 128),
        )

    nc.compile()
    return nc


def _get_nc():
    if "nc" not in _CACHE:
        _CACHE["nc"] = _build()
    return _CACHE["nc"]


def _make_in_maps(x, key_enc, val_enc, keys_t0, vals_t0, scales_t0,
                  keys_t1, vals_t1, scales_t1):
    import ml_dtypes
    bf = ml_dtypes.bfloat16
    f32 = np.float32

    def a(v, dt=None):
        v = np.asarray(v, dtype=np.float32)
        return v.astype(dt if dt is not None else bf)

    shared = {
        "key_enc": a(key_enc),
        "kt0T": a(np.asarray(keys_t0, dtype=np.float32).T),
        "vals_t0": a(vals_t0),
        "val_encT": a(np.asarray(val_enc, dtype=np.float32).T),
        "val_enc": a(val_enc),
        "keys_t1": a(keys_t1),
        "vt1T": a(np.asarray(vals_t1, dtype=np.float32).T),
        "s0": np.ascontiguousarray(
            np.asarray(scales_t0, dtype=f32).reshape(R0 // 128, 128).T),
        "s1": np.ascontiguousarray(
            np.asarray(scales_t1, dtype=f32).reshape(R1 // 128, 128).T),
    }
    x = np.asarray(x, dtype=np.float32)
    in_maps = []
    for c in range(NCORES):
        m = dict(shared)
        m["xT"] = np.ascontiguousarray(x[c * B:(c + 1) * B].T).astype(bf)
        in_maps.append(m)
    return in_maps


def _run(trace=False, **inputs):
    from concourse.bass_utils import run_bass_kernel_spmd
    nc = _get_nc()
    in_maps = _make_in_maps(**inputs)
    res = run_bass_kernel_spmd(nc, in_maps, core_ids=list(range(NCORES)),
                               trace=trace)
    y = np.concatenate(
        [np.asarray(res.results[c]["out"], dtype=np.float32).T
         for c in range(NCORES)], axis=0)
    return y, res


def kernel(**inputs) -> np.ndarray:
    y, _ = _run(trace=False, **inputs)
    return y


def _install_ntff_hook():
    """Make trace=True work under axon (antenv.axon_hooks is not shipped)."""
    import sys, types
    if "antenv.axon_hooks" in sys.modules:
        return
    mod = types.ModuleType("antenv.axon_hooks")
    state = {"hook": None}
    mod.set_axon_ntff_profile_hook = lambda h: state.__setitem__("hook", h)
    mod.get_axon_ntff_profile_hook = lambda: state["hook"]
    sys.modules["antenv.axon_hooks"] = mod
    from trn_agent_boot.trn_boot import _ntff_profile_via_ctypes
    mod.set_axon_ntff_profile_hook(
        _ntff_profile_via_ctypes("/opt/axon/libaxon_pjrt.so"))


def run_traced(**inputs):
    _install_ntff_hook()
    y, res = _run(trace=True, **inputs)
    return y, res.exec_time_ns


# revision 5
# speedup vs baseline: 1.0452x; 1.0452x over previous
"""Trainium2 Bass kernel for nn_AMM_76647986364863 (retrieval_knn).

Strategy: data-parallel over the batch dim of x across 8 NeuronCores
(64 rows/core); all tables/encoders replicated. Zero collectives.

Per-core compute keeps every activation transposed (features on the
partition dim, batch=64 on the free dim) so the whole chain is
weight-stationary matmuls with no on-chip transposes:
    qT      = key_enc^T x^T          (lhsT = key_enc)
    alphasT = keys_t0 qT             (lhsT = keys_t0^T, host-transposed)
    ybT     = vals_t0^T betasT       (lhsT = vals_t0)
    z0T     = val_enc ybT            (lhsT = val_enc^T, host-transposed)
    ISTA:  uT = val_enc^T zT (lhsT = val_enc);  gT = val_enc rT (lhsT = val_enc^T)
    t1:    alphas2T = keys_t1^T xT (lhsT = keys_t1); y1T = vals_t1 b2T (lhsT = vals_t1^T)

Since val_enc has exactly orthonormal columns (val_enc^T val_enc = I),
ISTA iteration 1 reduces to z1 = soft(z0): the first pair of big
matmuls is folded away (exact-math equivalent to the reference).

Matmuls run in bf16 (fp32 PSUM accumulate); weights are cast to bf16
and pre-tiled to the SBUF partition-major layout on the host, so every
weight DMA is fully contiguous per partition and chunked so the PE
chases the DMA stream. The z + g add of each ISTA step and the final
y = z5 + y1 add are folded into the PSUM accumulation groups via an
identity matmul (PE does the adds). soft(x) = x - clamp(x, -1, 1) via
a fused tensor_scalar(max,min) + tensor_tensor(sub) on DVE.
"""

import numpy as np

N = 2048      # x_dim
M = 2048      # y_dim
R0 = 1024
R1 = 1024
DK = 1024
DV = 1024
BATCH = 512
NCORES = 8
B = BATCH // NCORES            # 64 batch rows per core
ISTA_FULL_ITERS = 4            # reference does 5; iter 1 folds into soft(z0)
CHUNK_BYTES = 512 * 1024       # weight DMA chunk size

_CACHE = {}


def _build(warmup=True, ident_trick=True, out_chunks=True):
    from contextlib import ExitStack
    import concourse.tile as tile
    from concourse import bacc, mybir

    BF = mybir.dt.bfloat16
    F32 = mybir.dt.float32
    ALU = mybir.AluOpType

    nc = bacc.Bacc("TRN2", target_bir_lowering=False, debug=False,
                   num_devices=NCORES)

    def dp(name, shape, dt):
        return nc.dram_tensor(name, shape, dt, kind="ExternalInput").ap()

    # All weight/activation drams are host-pre-tiled to (128, t*F):
    # partition p, block t holds source row t*128+p.
    xT_d = dp("xT", [128, (N // 128) * B], BF)
    key_enc_d = dp("key_enc", [128, (N // 128) * DK], BF)
    kt0T_d = dp("kt0T", [128, (DK // 128) * R0], BF)
    vals_t0_d = dp("vals_t0", [128, (R0 // 128) * DV], BF)
    val_encT_d = dp("val_encT", [128, (DV // 128) * M], BF)
    val_enc_d = dp("val_enc", [128, (M // 128) * DV], BF)
    keys_t1_d = dp("keys_t1", [128, (N // 128) * R1], BF)
    vt1T_d = dp("vt1T", [128, (R1 // 128) * M], BF)
    s0_d = dp("s0", [128, R0 // 128], F32)
    s1_d = dp("s1", [128, R1 // 128], F32)
    ident_d = dp("ident", [128, 128], BF)
    out_d = nc.dram_tensor("out", [128, (M // 128) * B], F32,
                           kind="ExternalOutput").ap()

    with tile.TileContext(nc) as tc, ExitStack() as ctx:
        wres = ctx.enter_context(tc.tile_pool(name="wres", bufs=1))
        wstream = ctx.enter_context(tc.tile_pool(name="wstream", bufs=3))
        acts = ctx.enter_context(tc.tile_pool(name="acts", bufs=1))
        psum = ctx.enter_context(tc.tile_pool(name="psum", bufs=8, space="PSUM"))

        def act_tile(tag, nfree, dt=BF):
            return acts.tile([128, nfree], dt, tag=tag, name=tag)

        # ---- PE warm-up: junk matmuls to lift the HAM clock gate early ----
        if warmup:
            warm_sb = act_tile("warm", 512)
            nc.gpsimd.memset(warm_sb[:], 0.0)
            warm_ps = psum.tile([128, 512], F32, tag="ps", name="ps")
            for _ in range(6):
                nc.tensor.matmul(warm_ps[:], warm_sb[:, :128], warm_sb[:],
                                 start=True, stop=True)

        # ---- input / scale / const loads ----
        xT_sb = act_tile("xT", (N // 128) * B)
        nc.sync.dma_start(xT_sb[:], xT_d[:])
        s0_sb = act_tile("s0", R0 // 128, F32)
        nc.sync.dma_start(s0_sb[:], s0_d[:])
        s1_sb = act_tile("s1", R1 // 128, F32)
        nc.sync.dma_start(s1_sb[:], s1_d[:])
        id_sb = act_tile("ident", 128)
        nc.sync.dma_start(id_sb[:], ident_d[:])

        def load_w(pool, ap, tag):
            nfree = ap.shape[1]
            tl = pool.tile([128, nfree], BF, tag=tag, name=tag + "_w")
            step = max(1, CHUNK_BYTES // 256)   # bf16 cols per 2KB/partition... per chunk
            c = 0
            while c < nfree:
                e = min(nfree, c + step)
                nc.sync.dma_start(tl[:, c:e], ap[:, c:e])
                c = e
            return tl

        def mm_layer(w_tl, t, F, rhs_tl, consumer, add_from=None, post_add=None):
            """psum[m] = sum_k lhsT[k, m-block]^T @ rhs[k] (+ add_from[m]);
            consumer(m, psum_tile) evicts. w_tl free dim is t blocks of F."""
            if not ident_trick and add_from is not None:
                post_add, add_from = add_from, None
            for m in range(F // 128):
                ps = psum.tile([128, B], F32, tag="ps", name="ps")
                if add_from is not None:
                    nc.tensor.matmul(ps[:], id_sb[:],
                                     add_from[:, m * B:(m + 1) * B],
                                     start=True, stop=False)
                for k in range(t):
                    nc.tensor.matmul(
                        ps[:],
                        w_tl[:, k * F + m * 128: k * F + (m + 1) * 128],
                        rhs_tl[:, k * B:(k + 1) * B],
                        start=(k == 0 and add_from is None),
                        stop=(k == t - 1),
                    )
                consumer(m, ps)

        # ---- table 0: q = x @ key_enc ----
        w_key = load_w(wstream, key_enc_d, "w")
        qT_sb = act_tile("qT", (DK // 128) * B)

        def ev_q(m, ps):
            nc.vector.tensor_copy(qT_sb[:, m * B:(m + 1) * B], ps[:])

        mm_layer(w_key, N // 128, DK, xT_sb, ev_q)

        # ---- betasT = (keys_t0 @ qT) * s0 ----
        w_kt0 = load_w(wstream, kt0T_d, "w")
        betasT_sb = act_tile("betasT", (R0 // 128) * B)

        def ev_beta(m, ps):
            nc.vector.tensor_scalar_mul(
                betasT_sb[:, m * B:(m + 1) * B], ps[:], s0_sb[:, m:m + 1])

        mm_layer(w_kt0, DK // 128, R0, qT_sb, ev_beta)

        # ---- ybT = vals_t0^T @ betasT ----
        w_v0 = load_w(wstream, vals_t0_d, "w")
        ybT_sb = act_tile("ybT", (DV // 128) * B)

        def ev_yb(m, ps):
            nc.vector.tensor_copy(ybT_sb[:, m * B:(m + 1) * B], ps[:])

        mm_layer(w_v0, R0 // 128, DV, betasT_sb, ev_yb)

        # ---- resident ISTA weights ----
        w_veT = load_w(wres, val_encT_d, "val_encT")
        w_ve = load_w(wres, val_enc_d, "val_enc")

        zT_sb = act_tile("zT", (M // 128) * B)
        t_sb = None if ident_trick else act_tile("t", (M // 128) * B, F32)
        c_sb = act_tile("c", (M // 128) * B, F32)
        r_sb = act_tile("r", (DV // 128) * B)
        out_sb = act_tile("out", (M // 128) * B, F32)

        # ---- z0 = yb @ val_enc^T ; z1 = soft(z0) ----
        def ev_z0(m, ps):
            sl = slice(m * B, (m + 1) * B)
            nc.vector.tensor_scalar(c_sb[:, sl], ps[:], -1.0, 1.0,
                                    ALU.max, ALU.min)
            nc.vector.tensor_sub(zT_sb[:, sl], ps[:], c_sb[:, sl])

        mm_layer(w_veT, DV // 128, M, ybT_sb, ev_z0)

        # ---- ISTA full iterations: psum_g = z + g via identity matmul ----
        for it in range(ISTA_FULL_ITERS):

            def ev_r(m, ps):
                sl = slice(m * B, (m + 1) * B)
                nc.vector.tensor_sub(r_sb[:, sl], ybT_sb[:, sl], ps[:])

            mm_layer(w_ve, M // 128, DV, zT_sb, ev_r)

            def ev_soft(m, ps):
                sl = slice(m * B, (m + 1) * B)
                if ident_trick:
                    nc.vector.tensor_scalar(c_sb[:, sl], ps[:], -1.0, 1.0,
                                            ALU.max, ALU.min)
                    nc.vector.tensor_sub(zT_sb[:, sl], ps[:], c_sb[:, sl])
                else:
                    t_f = t_sb[:, sl]
                    nc.vector.tensor_add(t_f, zT_sb[:, sl], ps[:])
                    nc.vector.tensor_scalar(c_sb[:, sl], t_f, -1.0, 1.0,
                                            ALU.max, ALU.min)
                    nc.vector.tensor_sub(zT_sb[:, sl], t_f, c_sb[:, sl])

            mm_layer(w_veT, DV // 128, M, r_sb, ev_soft, add_from=zT_sb)

        # ---- table 1: y = z5 + (x @ keys_t1 * s1) @ vals_t1^T ----
        w_k1 = load_w(wstream, keys_t1_d, "w")
        b2_sb = act_tile("b2", (R1 // 128) * B)

        def ev_b2(m, ps):
            nc.vector.tensor_scalar_mul(
                b2_sb[:, m * B:(m + 1) * B], ps[:], s1_sb[:, m:m + 1])

        mm_layer(w_k1, N // 128, R1, xT_sb, ev_b2)

        w_v1T = load_w(wstream, vt1T_d, "w")
        OUT_CHUNK = 4  # m-tiles per output DMA

        def ev_y(m, ps):
            sl = slice(m * B, (m + 1) * B)
            if ident_trick:
                nc.vector.tensor_copy(out_sb[:, sl], ps[:])
            else:
                nc.vector.tensor_add(out_sb[:, sl], zT_sb[:, sl], ps[:])
            if out_chunks and (m + 1) % OUT_CHUNK == 0:
                osl = slice((m + 1 - OUT_CHUNK) * B, (m + 1) * B)
                nc.sync.dma_start(out_d[:, osl], out_sb[:, osl])

        mm_layer(w_v1T, R1 // 128, M, b2_sb, ev_y, add_from=zT_sb)
        if not out_chunks:
            nc.sync.dma_start(out_d[:], out_sb[:])

    nc.compile()
    return nc


import os
def _get_nc():
    if "nc" not in _CACHE:
        _CACHE["nc"] = _build(
            warmup=os.environ.get("K_WARMUP", "1") == "1",
            ident_trick=os.environ.get("K_IDENT", "1") == "1",
            out_chunks=os.environ.get("K_OUTCHUNKS", "1") == "1",
        )
    return _CACHE["nc"]


def _tile128(w):
    """(K, F) -> (128, (K//128)*F): partition-major pre-tiling."""
    K, F = w.shape
    t = K // 128
    return np.ascontiguousarray(
        w.reshape(t, 128, F).swapaxes(0, 1).reshape(128, t * F))


def _make_in_maps(x, key_enc, val_enc, keys_t0, vals_t0, scales_t0,
                  keys_t1, vals_t1, scales_t1):
    import ml_dtypes
    bf = ml_dtypes.bfloat16
    f32 = np.float32

    def prep(v, transpose=False):
        v = np.asarray(v, dtype=np.float32)
        if transpose:
            v = v.T
        return _tile128(v.astype(bf))

    shared = {
        "key_enc": prep(key_enc),
        "kt0T": prep(keys_t0, transpose=True),
        "vals_t0": prep(vals_t0),
        "val_encT": prep(val_enc, transpose=True),
        "val_enc": prep(val_enc),
        "keys_t1": prep(keys_t1),
        "vt1T": prep(vals_t1, transpose=True),
        "s0": np.ascontiguousarray(
            np.asarray(scales_t0, dtype=f32).reshape(R0 // 128, 128).T),
        "s1": np.ascontiguousarray(
            np.asarray(scales_t1, dtype=f32).reshape(R1 // 128, 128).T),
        "ident": np.eye(128, dtype=np.float32).astype(bf),
    }
    x = np.asarray(x, dtype=np.float32)
    in_maps = []
    for c in range(NCORES):
        m = dict(shared)
        m["xT"] = _tile128(np.ascontiguousarray(
            x[c * B:(c + 1) * B].T).astype(bf))
        in_maps.append(m)
    return in_maps


def _unpack_out(arr):
    """(128, 16*B) -> (B, 2048): inverse of the partition-major tiling."""
    t = M // 128
    return np.ascontiguousarray(
        np.asarray(arr, dtype=np.float32).reshape(128, t, B)
        .transpose(2, 1, 0).reshape(B, M))


def _run(trace=False, **inputs):
    from concourse.bass_utils import run_bass_kernel_spmd
    nc = _get_nc()
    in_maps = _make_in_maps(**inputs)
    res = run_bass_kernel_spmd(nc, in_maps, core_ids=list(range(NCORES)),
                               trace=trace)
    y = np.concatenate(
        [_unpack_out(res.results[c]["out"]) for c in range(NCORES)], axis=0)
    return y, res


def kernel(**inputs) -> np.ndarray:
    y, _ = _run(trace=False, **inputs)
    return y


def _install_ntff_hook():
    """Make trace=True work under axon (antenv.axon_hooks is not shipped)."""
    import sys, types
    if "antenv.axon_hooks" in sys.modules:
        return
    mod = types.ModuleType("antenv.axon_hooks")
    state = {"hook": None}
    mod.set_axon_ntff_profile_hook = lambda h: state.__setitem__("hook", h)
    mod.get_axon_ntff_profile_hook = lambda: state["hook"]
    sys.modules["antenv.axon_hooks"] = mod
    from trn_agent_boot.trn_boot import _ntff_profile_via_ctypes
    mod.set_axon_ntff_profile_hook(
        _ntff_profile_via_ctypes("/opt/axon/libaxon_pjrt.so"))


def run_traced(**inputs):
    _install_ntff_hook()
    y, res = _run(trace=True, **inputs)
    return y, res.exec_time_ns
